# revision 1
# baseline (speedup 1.0000x reference)
"""Trainium2 Bass kernel for the CapsuleLayer routing problem.

Strategy (pure data-parallel over 8 NeuronCores, batch-sharded):
  u = x @ W  via a 3-term fp16 hi/lo split on the TensorEngine
  (xh@Wh + xh@Wl + xl@Wh, fp32 PSUM accumulate).  fp16 pairs carry
  ~22 mantissa bits; the routing softmax amplifies matmul error
  ~1000-2000x at the batch tail, so nothing cheaper passes the 2e-2
  gate (measured: 2-term 0.36-0.62, fp16-stored-u 0.49).

  Routing restructured around a custom fused DVE op (CAPS_MAC_SCAN:
  out = running prefix sum of Src0*Src1).  Each of the four per-tile
  512-element routing passes (q1 = u.v1, s2' = sum_k e2 u_k,
  q2' = u.s2', s3' = sum_k e3 u_k) is ONE DVE instruction; the
  per-capsule / per-dim segment sums are recovered as differences of
  the prefix at segment boundaries (strided views, computed on the
  otherwise-idle GPSIMD engine against a zero lead column).  This
  replaces the baseline's 8 full multiply/reduce passes per tile
  (~7us/tile across DVE+GPSIMD) with ~2.4us of DVE + ~0.7us of GPSIMD.

Layout: batch rows on partitions, features (16 caps x 32 dims) on the
free dim.  x is staged transposed+tiled from the host so each x-tile is
directly usable as the matmul stationary operand (lhsT).
"""

import sys
import os

for _p in ("/opt/trn_rl_repo", "/root/.axon_site/_ro/trn_rl_repo"):
    if os.path.isdir(_p) and _p not in sys.path:
        sys.path.insert(0, _p)
        break

import numpy as np
import ml_dtypes

import concourse.bass as bass
import concourse.bacc as bacc
import concourse.mybir as mybir
from concourse import tile
from concourse import dve_ops as dops
from concourse.dve_spec import Spec, Src0, Src1, scan, lower, AluOp
from concourse.dve_uop import DveOpSpec
from concourse.dve_ops import DveOp
from concourse.bass_utils import run_bass_kernel_spmd

BF16 = mybir.dt.bfloat16
F16 = mybir.dt.float16
F32 = mybir.dt.float32
NP_BF16 = ml_dtypes.bfloat16

NCORES = 8
B = 32768
K = 512
CAPS = 16
D = 32
ND = CAPS * D          # 512
BS = B // NCORES       # 4096 rows per core
P = 128                # partitions per tile
TILES = BS // P        # 32
G = 8                  # tiles per beta group
GROUPS = [(0, 2), (2, 6), (8, 8), (16, 8), (24, 8)]
KCH = K // P           # 4 contraction chunks
NSLOT = 12             # prefix-buffer ring depth

AX = mybir.AxisListType.X
OP_ADD = mybir.AluOpType.add
OP_SUB = mybir.AluOpType.subtract
OP_MUL = mybir.AluOpType.mult
OP_MAX = mybir.AluOpType.max
FN = mybir.ActivationFunctionType


def _patch_act_tables():
    """Make the act-table-load pass resolve Exp and Ln to the combined
    natural_log_exp_and_others set so one table load serves the whole
    kernel (first-fit would otherwise alternate exp<->ln sets, ~2.7us per
    switch).  Indices must stay aligned with act_info.json, so only the
    function-membership sets are edited."""
    from concourse import hw_specs
    if getattr(hw_specs, "_capsule_patched", False):
        return
    orig = hw_specs.get_activation_tables

    def patched(module_arch):
        tables = {k: set(v) for k, v in orig(module_arch).items()}
        comb = "natural_log_exp_and_others"
        if comb in tables:
            for name, fns in tables.items():
                if name != comb:
                    fns.discard(FN.Exp)
                    fns.discard(FN.Ln)
        return tables

    import functools
    patched_cached = functools.cache(patched)
    hw_specs.get_activation_tables = patched_cached
    bacc.get_activation_tables = patched_cached
    hw_specs._capsule_patched = True


def _mac_scan_ref(in0, in1, c0, c1, c2):
    a = np.asarray(in0, np.float32) * np.asarray(in1, np.float32)
    flat = a.reshape(a.shape[0], -1)
    out = np.cumsum(flat.astype(np.float64), axis=1).astype(np.float32)
    return out.reshape(a.shape)


def _register_mac_scan():
    """Register the fused multiply+prefix-sum custom DVE op (documented
    per-NEFF DVE-table mechanism; one added OPS row)."""
    name = "CAPS_MAC_SCAN"
    for op in dops.OPS:
        if op.name == name:
            return op
    spec = Spec(body=scan(AluOp.ADD, Src0 * Src1), reference=_mac_scan_ref)
    row = dops._CUSTOM_DVE_ROW_BASE + len(dops.OPS)
    assert row < 0x20, "custom DVE opcode rows exhausted"
    shas = {}
    for ver in ("v3", "v4"):
        s = DveOpSpec(name=name, opcode=row, uops=lower(spec, ver=ver),
                      rd1_en=True)
        shas[ver] = s.sha(ver)
    op = DveOp(name, spec, subdim=False, uops_sha=shas)
    dops.OPS.append(op)
    dops.CUSTOM_DVE_SPECS[name] = spec
    dops._SUB_OPCODE_FOR_NAME[name] = row
    return op


def _build_program():
    _patch_act_tables()
    mac_scan = _register_mac_scan()
    nc = bacc.Bacc("TRN2", target_bir_lowering=False)

    xTh = nc.declare_dram_parameter("xh", [TILES, P, K], F16, isOutput=False)
    xTl = nc.declare_dram_parameter("xl", [TILES, P, K], F16, isOutput=False)
    WPK = 2 * (ND + D)  # 1088 packed weight cols per chunk
    Wpk = nc.declare_dram_parameter("Wpk", [KCH, P, WPK], F16, isOutput=False)
    vout = nc.declare_dram_parameter("v", [BS, D], F32, isOutput=True)
    vview = vout.ap().rearrange("(t p) d -> t p d", p=P)

    with tile.TileContext(nc) as tc:
        with (
            tc.tile_pool(name="wpool", bufs=1) as wpool,
            tc.tile_pool(name="xpool", bufs=5) as xpool,
            tc.tile_pool(name="upsum", bufs=4, space="PSUM") as upsum,
            tc.tile_pool(name="spsum", bufs=4, space="PSUM") as spsum,
            tc.tile_pool(name="upool", bufs=20) as upool,
            tc.tile_pool(name="prefpool", bufs=1) as prefpool,
            tc.tile_pool(name="s1pool", bufs=16) as s1pool,
            tc.tile_pool(name="gpool", bufs=4) as gpool,
            tc.tile_pool(name="spool", bufs=16) as spool,
        ):
            # --- constants: per-chunk DMAs so chunk-0 weights land first
            # and the first matmuls start without waiting for the rest ---
            wchunks = []
            for c in range(KCH):
                wc = wpool.tile([P, WPK], F16, tag=f"wall{c}", name="wc")
                nc.sync.dma_start(wc[:], Wpk.ap()[c])
                wchunks.append(wc)
            Wh = [wchunks[c][:, 0:ND] for c in range(KCH)]
            Wl = [wchunks[c][:, ND:2 * ND] for c in range(KCH)]
            Wsh = [wchunks[c][:, 2 * ND:2 * ND + D] for c in range(KCH)]
            Wsl = [wchunks[c][:, 2 * ND + D:2 * ND + 2 * D] for c in range(KCH)]

            # --- PE pstate warm-up: junk matmuls on zeroed SBUF keep the
            # TensorEngine continuously busy through the initial DMA wait,
            # so the first real matmuls run at full clock (the PE needs
            # ~3us of uninterrupted work to ramp to peak) ---
            wu_in = wpool.tile([P, P], F16, tag="wu_in", name="wu_in")
            wu_w = wpool.tile([P, ND], F16, tag="wu_w", name="wu_w")
            nc.vector.memset(wu_in[:], 0.0)
            nc.vector.memset(wu_w[:], 0.0)
            wu_ps = upsum.tile([P, ND], F32, tag="u_ps", name="wu_ps")
            for r in range(7):
                nc.tensor.matmul(wu_ps[:], wu_in[:], wu_w[:],
                                 start=(r == 0), stop=(r == 6))

            # --- prefix-buffer ring: lead column stays 0 forever ---
            pslots = []
            for i in range(NSLOT):
                pt = prefpool.tile([P, ND + 1], F32, tag=f"pref{i}")
                nc.vector.memset(pt[:, 0:1], 0.0)
                pslots.append(pt)
            pctr = [0]

            def scan_pass(u_sb, in1_ap, order):
                """One fused MAC+prefix pass over u; returns (minuend,
                subtrahend) strided views whose difference is the segment
                sums (16 q-values for order='q', 32 s-values for 's')."""
                pt = pslots[pctr[0] % NSLOT]
                pctr[0] += 1
                if order == "q":
                    out_ap = pt[:, 1:ND + 1].rearrange("p (k d) -> p k d", d=D)
                    in0_ap = u_sb[:].rearrange("p (k d) -> p k d", d=D)
                    step = D
                    nseg = CAPS
                else:
                    out_ap = pt[:, 1:ND + 1].rearrange("p (d k) -> p d k", k=CAPS)
                    in0_ap = u_sb[:].rearrange("p (k d) -> p d k", d=D)
                    step = CAPS
                    nseg = D
                nc.vector._custom_dve(
                    mac_scan, out=out_ap, in0=in0_ap, in1=in1_ap)
                flat = pt[:].rearrange("p a -> p a")
                minu = flat[:, step::step]
                subt = flat[:, 0::step][:, 0:nseg]
                return minu, subt

            def issue_dma(st):
                """Prefetch the group's split x tiles (two DMAs)."""
                T0, GS = st["T0"], st["GS"]
                xgh = st["xgh"] = xpool.tile([P, GS * K], F16, tag="xgh",
                                             name="xgh")
                nc.sync.dma_start(
                    xgh[:].rearrange("p (t f) -> p t f", t=GS),
                    xTh[T0:T0 + GS].rearrange("t p f -> p t f"),
                )
                xgl = st["xgl"] = xpool.tile([P, GS * K], F16, tag="xgl",
                                             name="xgl")
                nc.sync.dma_start(
                    xgl[:].rearrange("p (t f) -> p t f", t=GS),
                    xTl[T0:T0 + GS].rearrange("t p f -> p t f"),
                )

            def phase1(st):
                """Group state st: dict with T0, GS; fills buffers."""
                T0, GS = st["T0"], st["GS"]
                for tag, width in (
                    ("q1g", CAPS), ("l2g", CAPS), ("e2g", CAPS),
                    ("q2g", CAPS), ("l3g", CAPS), ("e3g", CAPS),
                    ("s3g", D), ("sqg", D), ("vg", D),
                ):
                    st[tag] = gpool.tile([P, GS * width], F32, tag=tag, name=tag)
                for tag in ("nu1", "gam1", "m2", "r2", "sig2", "nu2", "del2",
                            "m3", "r3", "sig3", "nu3", "alp3", "tmpa", "tmpb"):
                    st[tag] = gpool.tile([P, GS], F32, tag=tag, name=tag)
                st["prodg"] = gpool.tile([P, GS * CAPS], F32, tag="prodg",
                                         name="prodg")
                q1g = st["q1g"]

                u_tiles = st["u_tiles"] = []
                s1_tiles = st["s1_tiles"] = []
                xgh, xgl = st["xgh"], st["xgl"]
                # ---- matmul + q1 per tile ----
                for t in range(GS):
                    u_ps = upsum.tile([P, ND], F32, tag="u_ps")
                    s_ps = spsum.tile([P, D], F32, tag="s_ps")
                    for c in range(KCH):
                        xh = xgh[:, t * K + c * P: t * K + (c + 1) * P]
                        xl = xgl[:, t * K + c * P: t * K + (c + 1) * P]
                        first = c == 0
                        last = c == KCH - 1
                        # u += xh@Wh + xh@Wl + xl@Wh   (fp16 hi/lo split)
                        nc.tensor.matmul(u_ps[:], xh, Wh[c],
                                         start=first, stop=False)
                        nc.tensor.matmul(u_ps[:], xh, Wl[c],
                                         start=False, stop=False)
                        nc.tensor.matmul(s_ps[:], xh, Wsh[c],
                                         start=first, stop=False)
                        nc.tensor.matmul(s_ps[:], xh, Wsl[c],
                                         start=False, stop=False)
                        nc.tensor.matmul(u_ps[:], xl, Wh[c],
                                         start=False, stop=last)
                        nc.tensor.matmul(s_ps[:], xl, Wsh[c],
                                         start=False, stop=last)

                    u_sb = upool.tile([P, ND], F32, tag="u_sb")
                    nc.scalar.copy(u_sb[:], u_ps[:])
                    s1_sb = s1pool.tile([P, D], F32, tag="s1_sb")
                    nc.scalar.copy(s1_sb[:], s_ps[:])
                    u_tiles.append(u_sb)
                    s1_tiles.append(s1_sb)

                    # q1 = sum_d u * bcast_k(s1): one fused scan + diff
                    minu, subt = scan_pass(
                        u_sb,
                        s1_sb[:].unsqueeze(1).broadcast_to([P, CAPS, D]),
                        "q")
                    nc.gpsimd.tensor_tensor(
                        q1g[:, t * CAPS:(t + 1) * CAPS], minu, subt, OP_SUB)

            def beta1(st):
                GS = st["GS"]
                q1g, l2g, e2g = st["q1g"], st["l2g"], st["e2g"]
                nu1, gam1, m2, r2 = st["nu1"], st["gam1"], st["m2"], st["r2"]
                tmpa, tmpb = st["tmpa"], st["tmpb"]
                q1v = q1g[:].rearrange("p (t k) -> p t k", t=GS)
                nc.vector.tensor_reduce(nu1[:], q1v, AX, OP_ADD)  # = 16*nu1
                # gamma1 = exp(0.5*ln(nu1)) / (1 + nu1);  ln(nu1) = ln(sum/16)
                nc.scalar.activation(tmpa[:], nu1[:], FN.Ln, scale=1.0 / CAPS)
                nc.scalar.activation(tmpa[:], tmpa[:], FN.Exp, scale=0.5)  # sqrt(nu1)
                nc.vector.tensor_scalar(tmpb[:], nu1[:], 1.0 / CAPS, 1.0, OP_MUL, OP_ADD)
                nc.vector.reciprocal(tmpb[:], tmpb[:])
                nc.vector.tensor_tensor(gam1[:], tmpa[:], tmpb[:], OP_MUL)
                # l2 = gamma1 * q1 ; m2 = max_k l2 ; e2 = exp(l2 - m2)
                g1b = gam1[:].unsqueeze(2).broadcast_to([P, GS, CAPS])
                nc.vector.tensor_tensor(
                    l2g[:].rearrange("p (t k) -> p t k", t=GS), q1v, g1b, OP_MUL)
                nc.vector.tensor_reduce(
                    m2[:], l2g[:].rearrange("p (t k) -> p t k", t=GS), AX, OP_MAX)
                nc.vector.tensor_tensor(
                    l2g[:].rearrange("p (t k) -> p t k", t=GS),
                    l2g[:].rearrange("p (t k) -> p t k", t=GS),
                    m2[:].unsqueeze(2).broadcast_to([P, GS, CAPS]),
                    OP_SUB,
                )
                nc.scalar.activation(e2g[:], l2g[:], FN.Exp)
                nc.vector.tensor_reduce(
                    r2[:], e2g[:].rearrange("p (t k) -> p t k", t=GS), AX, OP_ADD)
                nc.vector.reciprocal(r2[:], r2[:])

            def phase2(st):
                GS = st["GS"]
                e2g, q2g = st["e2g"], st["q2g"]
                u_tiles = st["u_tiles"]
                for t in range(GS):
                    u_sb = u_tiles[t]
                    e2s = e2g[:, t * CAPS:(t + 1) * CAPS]
                    # s2' = sum_k e2_k u_k: one fused scan (d-major) + diff
                    minu, subt = scan_pass(
                        u_sb,
                        e2s.unsqueeze(1).broadcast_to([P, D, CAPS]),
                        "s")
                    s2p = spool.tile([P, D], F32, tag="s2p")
                    nc.gpsimd.tensor_tensor(s2p[:], minu, subt, OP_SUB)
                    # q2' = u . s2': one fused scan (k-major) + diff
                    minu, subt = scan_pass(
                        u_sb,
                        s2p[:].unsqueeze(1).broadcast_to([P, CAPS, D]),
                        "q")
                    nc.gpsimd.tensor_tensor(
                        q2g[:, t * CAPS:(t + 1) * CAPS], minu, subt, OP_SUB)

            def beta2(st):
                GS = st["GS"]
                q2g, e2g, l2g, l3g, e3g = (st["q2g"], st["e2g"], st["l2g"],
                                           st["l3g"], st["e3g"])
                prodg, sig2, nu2, del2 = (st["prodg"], st["sig2"], st["nu2"],
                                          st["del2"])
                m3, r3, r2 = st["m3"], st["r3"], st["r2"]
                tmpa, tmpb = st["tmpa"], st["tmpb"]
                q2v = q2g[:].rearrange("p (t k) -> p t k", t=GS)
                nc.vector.tensor_tensor(
                    prodg[:].rearrange("p (t k) -> p t k", t=GS),
                    e2g[:].rearrange("p (t k) -> p t k", t=GS), q2v, OP_MUL)
                nc.vector.tensor_reduce(
                    sig2[:], prodg[:].rearrange("p (t k) -> p t k", t=GS), AX, OP_ADD)
                nc.vector.tensor_tensor(tmpa[:], r2[:], r2[:], OP_MUL)
                nc.vector.tensor_tensor(nu2[:], sig2[:], tmpa[:], OP_MUL)
                nc.scalar.activation(tmpa[:], nu2[:], FN.Ln)
                nc.scalar.activation(tmpa[:], tmpa[:], FN.Exp, scale=0.5)
                nc.vector.tensor_scalar(tmpb[:], nu2[:], 1.0, 1.0, OP_MUL, OP_ADD)
                nc.vector.reciprocal(tmpb[:], tmpb[:])
                nc.vector.tensor_tensor(tmpa[:], tmpa[:], tmpb[:], OP_MUL)  # gamma2
                nc.vector.tensor_tensor(del2[:], tmpa[:], r2[:], OP_MUL)
                # l3 = l2 + del2 * q2'
                nc.vector.tensor_tensor(
                    prodg[:].rearrange("p (t k) -> p t k", t=GS),
                    q2v,
                    del2[:].unsqueeze(2).broadcast_to([P, GS, CAPS]),
                    OP_MUL,
                )
                nc.vector.tensor_tensor(l3g[:], l2g[:], prodg[:], OP_ADD)
                nc.vector.tensor_reduce(
                    m3[:], l3g[:].rearrange("p (t k) -> p t k", t=GS), AX, OP_MAX)
                nc.vector.tensor_tensor(
                    l3g[:].rearrange("p (t k) -> p t k", t=GS),
                    l3g[:].rearrange("p (t k) -> p t k", t=GS),
                    m3[:].unsqueeze(2).broadcast_to([P, GS, CAPS]),
                    OP_SUB,
                )
                nc.scalar.activation(e3g[:], l3g[:], FN.Exp)
                nc.vector.tensor_reduce(
                    r3[:], e3g[:].rearrange("p (t k) -> p t k", t=GS), AX, OP_ADD)
                nc.vector.reciprocal(r3[:], r3[:])

            def phase3(st):
                GS = st["GS"]
                e3g, s3g = st["e3g"], st["s3g"]
                u_tiles = st["u_tiles"]
                for t in range(GS):
                    u_sb = u_tiles[t]
                    e3s = e3g[:, t * CAPS:(t + 1) * CAPS]
                    minu, subt = scan_pass(
                        u_sb,
                        e3s.unsqueeze(1).broadcast_to([P, D, CAPS]),
                        "s")
                    nc.gpsimd.tensor_tensor(
                        s3g[:, t * D:(t + 1) * D], minu, subt, OP_SUB)

            def beta3(st):
                T0, GS = st["T0"], st["GS"]
                s3g, sqg, vg = st["s3g"], st["sqg"], st["vg"]
                sig3, nu3, alp3, r3 = st["sig3"], st["nu3"], st["alp3"], st["r3"]
                tmpa, tmpb = st["tmpa"], st["tmpb"]
                nc.vector.tensor_tensor(sqg[:], s3g[:], s3g[:], OP_MUL)
                nc.vector.tensor_reduce(
                    sig3[:], sqg[:].rearrange("p (t d) -> p t d", t=GS), AX, OP_ADD)
                nc.vector.tensor_tensor(tmpa[:], r3[:], r3[:], OP_MUL)
                nc.vector.tensor_tensor(nu3[:], sig3[:], tmpa[:], OP_MUL)
                nc.scalar.activation(tmpa[:], nu3[:], FN.Ln)
                nc.scalar.activation(tmpa[:], tmpa[:], FN.Exp, scale=0.5)
                nc.vector.tensor_scalar(tmpb[:], nu3[:], 1.0, 1.0, OP_MUL, OP_ADD)
                nc.vector.reciprocal(tmpb[:], tmpb[:])
                nc.vector.tensor_tensor(tmpa[:], tmpa[:], tmpb[:], OP_MUL)  # gamma3
                nc.vector.tensor_tensor(alp3[:], tmpa[:], r3[:], OP_MUL)
                nc.vector.tensor_tensor(
                    vg[:].rearrange("p (t d) -> p t d", t=GS),
                    s3g[:].rearrange("p (t d) -> p t d", t=GS),
                    alp3[:].unsqueeze(2).broadcast_to([P, GS, D]),
                    OP_MUL,
                )
                nc.sync.dma_start(
                    vview[T0:T0 + GS].rearrange("t p d -> p t d"),
                    vg[:].rearrange("p (t d) -> p t d", t=GS))

            # ---- software-pipelined emission: group g+1's phase-1 fills
            # the engines while group g's beta chains run ----
            states = [{"T0": T0, "GS": GS} for (T0, GS) in GROUPS]
            ng = len(states)
            issue_dma(states[0])
            issue_dma(states[1])
            issue_dma(states[2])
            phase1(states[0])
            beta1(states[0])
            for g in range(1, ng):
                if g + 2 < ng:
                    issue_dma(states[g + 2])
                phase2(states[g - 1])
                phase1(states[g])
                beta2(states[g - 1])
                phase3(states[g - 1])
                beta1(states[g])
                beta3(states[g - 1])
            phase2(states[ng - 1])
            beta2(states[ng - 1])
            phase3(states[ng - 1])
            beta3(states[ng - 1])

    nc.compile()
    return nc


_PROG_CACHE = {}


def _get_program():
    if "nc" not in _PROG_CACHE:
        _PROG_CACHE["nc"] = _build_program()
    return _PROG_CACHE["nc"]


def _split16(a):
    hi = a.astype(np.float16)
    lo = (a - hi.astype(np.float32)).astype(np.float16)
    return hi, lo


def _stage_inputs(x, W):
    x = np.ascontiguousarray(x, dtype=np.float32)
    W = np.ascontiguousarray(W, dtype=np.float32)
    Ws = W.reshape(K, CAPS, D).mean(axis=1, dtype=np.float32)
    Whh, Wll = _split16(W.reshape(KCH, P, ND))
    Wsh, Wsl = _split16(Ws.reshape(KCH, P, D))
    Wpk = np.ascontiguousarray(
        np.concatenate([Whh, Wll, Wsh, Wsl], axis=2))

    in_maps = []
    for core in range(NCORES):
        xs = x[core * BS:(core + 1) * BS]
        # lhsT tile layout: [tile, kappa_in_chunk(P), (chunk, j)]
        xt = np.ascontiguousarray(
            xs.reshape(TILES, P, KCH, P).transpose(0, 3, 2, 1)
        ).reshape(TILES, P, K)
        xh, xl = _split16(xt)
        in_maps.append({"xh": xh, "xl": xl, "Wpk": Wpk})
    return in_maps


def kernel(x, W, _trace=False, _trace_kwargs=None):
    nc = _get_program()
    in_maps = _stage_inputs(np.asarray(x), np.asarray(W))
    res = run_bass_kernel_spmd(
        nc, in_maps, list(range(NCORES)), trace=_trace,
        **(_trace_kwargs or {}),
    )
    out = np.concatenate(
        [np.asarray(res.results[i]["v"], dtype=np.float32) for i in range(NCORES)],
        axis=0,
    )
    if _trace:
        kernel._last_results = res
    return out

